# revision 13
# baseline (speedup 1.0000x reference)
"""Embedding-similarity group merge on 8 Trainium2 NeuronCores.

Strategy (v9: fp8 DoubleRow + 4-slot psum ring scan)
----------------------------------------------------
The heavy part of the reference (Embeddings._fast_predict) is the blocked
cosine-similarity score computation V @ V.T (16384 x 16384 x 256).  The
transitive group-merge that follows is sequential and path-dependent but only
touches the ~10k above-threshold pairs, so it is replayed exactly on host.

Device work per core (SPMD, identical program; per-core behaviour comes only
from the vq input = that core's 16 interleaved query i-tiles):

* Matmul in fp8e4 with perf_mode=DoubleRow: the PE array virtualizes to
  128x256, so the full D=256 contraction happens in ONE matmul per
  [128 x 512] output tile at 2 fp8 elements/cycle.  With both operands
  rounded to fp8e4, thresholding at thr - EPS yields a guaranteed superset
  of the true fp32 matches (EPS validated empirically).
* The psum scan is the wall: only Vector and Scalar can read PSUM (DVE
  0.96 GHz, ACT 1.2 GHz, 1 elem/cycle/lane).  The full 8-bank psum is a
  ring of four 2-bank slots (1024 cols each), one scan op per unit:
    - Scalar units: Sign(sims - thr') -> u8 mask [128, 1024], staged in
      pairs, drained by DMA alternating over the Sync/GpSimd rings
    - Vector units: segmented reduce_max -> [128, 8] f32 (128-col
      segments); host re-expands flagged segments
  With four slots in flight, a slot's refill matmuls + semaphore
  roundtrip (~2us) hide behind the other slots' scans.  Engine
  assignment is greedily balanced (~1.04 + ~0.84 = 1.88 cols/ns/core).
  2048-col ops were tried and are a net loss: a bank is held from its
  refill until its scan op completes, and by Little's law the 8-bank
  psum cannot cover the doubled residency.
* Input DMA: descriptors stream FIFO per DGE ring at ~120-270 GB/s, so
  the vt stream is issued in consumption-sized chunks on the GpSimd
  SWDGE ring (b1, b2-3, b4-7, b8-11, b12-15) while block 0 rides the
  Sync ring (split in column halves so the first matmul starts one
  half-transfer earlier) and vq the Act ring -- three rings in
  parallel, earliest-needed data first.
* Slot s scans col-blocks jb = s..15 (uniform across cores thanks to the
  i-tile interleave: global i-tile of (core c, slot s) is 8s+c or 8s+7-c,
  both with t//8 == s), emitted in jb-ascending rounds so consumption
  follows the vt stream.
* HAM warmup: 8 dependency-free matmuls on zeroed scratch bridge the PE
  clock gate (K=4/8 half rate by default, needs ~3.4us sustained activity)
  across the DMA head.

Host: expands/gathers candidate pairs, recomputes their sims exactly in
fp32, applies the reference's column mask, and replays the reference's
sequential batch/row merge to produce bit-identical group ids.
"""

import sys

if "/opt/trn_rl_repo" not in sys.path:
    sys.path.insert(0, "/opt/trn_rl_repo")

import numpy as np
import ml_dtypes

import concourse.bass as bass
import concourse.tile as tile
from concourse import bacc, mybir
from concourse.bass_utils import run_bass_kernel_spmd

N_CORES = 8
D = 256                     # embedding dim (2 chunks of 128 on partitions)
EPS = 0.030                 # fp8e4 guard band (measured max err 0.023 @67M pairs)
I_TILE = 128                # query rows per slot
UNIT = 1024                 # cols per unit = one psum ring slot (2 banks)
SEG = 128                   # reduce-max segment width (cols)
SLOTS = 16                  # i-tiles per core
RING = 4                    # psum ring slots (2 banks each = full 8-bank psum)
MASK_BATCH = 2              # scalar units per mask stage DMA
MX_BATCH = 8                # vector units per mx stage DMA
N_WARMUP = 8                # HAM warmup matmuls

V_COST = (147.0 + UNIT) / 0.96        # DVE cost per 1024-col reduce op
S_COST1 = (312.0 + UNIT) / 1.2        # ACT cost per 1024-col mask op

_BUILD_CACHE: dict = {}
LAST_EXEC_NS = None         # set when kernel() runs with TRACE=True
TRACE = False


def _itile_for_slot(c: int, s: int) -> int:
    """Global i-tile handled by core c in slot s.

    k, r = divmod(s, 2); r=0 -> 16k + c, r=1 -> 16k + 15 - c.  Either way
    t // 8 == s for every core, so the unit layout (and hence the compiled
    program) is identical across cores."""
    k, r = divmod(s, 2)
    return 16 * k + (c if r == 0 else 15 - c)


def _unit_layout(n_blocks: int):
    """Units (s, jb) in jb-ascending rounds: slot s covers col-blocks
    jb = s..n_blocks-1 (upper-triangle superset at 1024-col granularity)."""
    return [(s, jb) for jb in range(n_blocks) for s in range(min(jb + 1, SLOTS))]


def _assign_engines(units):
    """Greedy balance of scan cost: one 1024-col op per unit.

    Larger (2048-col) scalar ops were tried and are a net loss: a bank is
    held from its refill matmul until its scan op completes, so by Little's
    law the 8-bank psum caps throughput at ~4096 cols per avg bank cycle --
    doubling op duration inflates residency past what the psum can cover.
    1024-col ops balance per-op init overhead (DVE 147 cyc, ACT 312 cyc)
    against residency."""
    kinds = []
    v_acc = s_acc = 0.0
    for _ in units:
        if v_acc + V_COST <= s_acc + S_COST1:
            kinds.append("max")
            v_acc += V_COST
        else:
            kinds.append("mask")
            s_acc += S_COST1
    return kinds


def _ensure_ntff_hook():
    """Register the axon NTFF-profile hook (test/trace path only).

    The agent image's ``antenv`` lacks ``axon_hooks``, so ``trn_boot.boot``
    silently skips hook registration and ``bass_utils`` would crash on the
    import. Seed ``sys.modules['antenv.axon_hooks']`` with a stub wired to
    the ctypes hook so ``trace=True`` yields real NTFF profiles."""
    import types
    if "antenv.axon_hooks" in sys.modules:
        return
    try:
        from trn_agent_boot.trn_boot import _ntff_profile_via_ctypes
        hook = _ntff_profile_via_ctypes("/opt/axon/libaxon_pjrt.so")
    except Exception:
        hook = None
    mod = types.ModuleType("antenv.axon_hooks")
    mod._HOOK = hook
    mod.get_axon_ntff_profile_hook = lambda: mod._HOOK
    mod.set_axon_ntff_profile_hook = lambda h: setattr(mod, "_HOOK", h)
    sys.modules["antenv.axon_hooks"] = mod


def _build_program(n_cols: int, thr_dev: float) -> bass.Bass:
    """One SPMD program, identical across cores.

    Inputs (per core):
      vt [2, 128, n_cols] fp8e4 -- V.T split into two 128-row d-chunks
      vq [2, 128, 2048] fp8e4   -- this core's 16 i-tiles of query columns
    Outputs:
      mask [nb_mask, 128, MASK_BATCH, 1024] u8 -- scalar candidate masks
      mx   [nb_max, 128, MX_BATCH, 8] f32      -- vector segment maxes
    """
    n_blocks = n_cols // UNIT
    units = _unit_layout(n_blocks)
    kinds = _assign_engines(units)
    rows = SLOTS * I_TILE
    nseg_h = 512 // SEG                  # segments per 512-col bank = 4
    n_mask = sum(1 for k in kinds if k == "mask")
    n_max = len(kinds) - n_mask
    nb_mask = (n_mask + MASK_BATCH - 1) // MASK_BATCH
    nb_max = (n_max + MX_BATCH - 1) // MX_BATCH

    nc = bacc.Bacc(None, target_bir_lowering=False)
    vt_d = nc.declare_dram_parameter("vt", [2, 128, n_cols], mybir.dt.float8e4, isOutput=False)
    vq_d = nc.declare_dram_parameter("vq", [2, 128, rows], mybir.dt.float8e4, isOutput=False)
    mask_d = nc.declare_dram_parameter(
        "mask", [max(nb_mask, 1), I_TILE, MASK_BATCH, UNIT],
        mybir.dt.uint8, isOutput=True)
    mx_d = nc.declare_dram_parameter(
        "mx", [max(nb_max, 1), I_TILE, MX_BATCH, 2 * nseg_h],
        mybir.dt.float32, isOutput=True)

    with tile.TileContext(nc) as tc:
        with (
            tc.tile_pool(name="vt", bufs=1) as vt_pool,
            tc.tile_pool(name="vq", bufs=1) as vq_pool,
            tc.tile_pool(name="psum", bufs=1, space="PSUM") as psum_pool,
            tc.tile_pool(name="stage", bufs=8) as stage_pool,
            tc.tile_pool(name="mxs", bufs=4) as mx_pool,
        ):
            vt_sb = vt_pool.tile([128, 2, n_cols], mybir.dt.float8e4)
            vq_sb = vq_pool.tile([128, 2, rows], mybir.dt.float8e4)
            bias_t = vq_pool.tile([128, 1], mybir.dt.float32)
            scratch = vq_pool.tile([128, 2, 512], mybir.dt.float8e4)
            # vector owns the tiny init memsets so gpsimd can start issuing
            # the vt stream immediately
            nc.vector.memset(scratch, 0)
            nc.vector.memset(bias_t, -thr_dev)
            # Full 8-bank psum as a ring of four 2-bank slots.  Subtile
            # dependency tracking orders refills against the slot's last
            # scan; with >=3 regions in flight the refill+sem latency
            # (~2us) hides behind the other slots' scans.
            ps = psum_pool.tile([128, 8, nseg_h, SEG], mybir.dt.float32)

            # HAM warmup on bank 7 (first real user is unit 3, itself gated
            # on the vt stream).
            for _ in range(N_WARMUP):
                nc.tensor.matmul(
                    ps[:, 7], lhsT=scratch[:, :, :128], rhs=scratch,
                    start=True, stop=True,
                    perf_mode=mybir.MatmulPerfMode.DoubleRow,
                )

            # Input streams on three parallel DGE rings, earliest-needed
            # first: b0 on Sync (whose ring later carries the outputs), vq
            # on Act, b1..b15 in consumption-sized FIFO chunks on GpSimd.
            nc.scalar.dma_start(out=vq_sb[:, :, :], in_=vq_d[:, :, :])
            nc.sync.dma_start(out=vt_sb[:, :, :512], in_=vt_d[:, :, :512])
            nc.sync.dma_start(
                out=vt_sb[:, :, 512:UNIT], in_=vt_d[:, :, 512:UNIT])
            for lo, hi in ((1, 2), (2, 4), (4, 8), (8, 12), (12, 16)):
                nc.gpsimd.dma_start(
                    out=vt_sb[:, :, lo * UNIT:hi * UNIT],
                    in_=vt_d[:, :, lo * UNIT:hi * UNIT])

            i_mask = i_max = 0
            stage = mxt = None
            for u, (s, jb) in enumerate(units):
                ts = slice(s * I_TILE, (s + 1) * I_TILE)
                r = u % RING
                for h in range(2):
                    j0 = jb * UNIT + h * 512
                    nc.tensor.matmul(
                        ps[:, 2 * r + h],
                        lhsT=vq_sb[:, :, ts], rhs=vt_sb[:, :, j0:j0 + 512],
                        start=True, stop=True,
                        perf_mode=mybir.MatmulPerfMode.DoubleRow,
                    )
                if kinds[u] == "mask":
                    if stage is None:
                        stage = stage_pool.tile(
                            [128, MASK_BATCH, 2, nseg_h, SEG], mybir.dt.uint8)
                    b = i_mask % MASK_BATCH
                    # Sign(sims - thr'): +1 above threshold; 0/255 otherwise
                    # (f32->u8 of -1 may wrap). Host treats ==1 as candidate.
                    nc.scalar.activation(
                        stage[:, b], ps[:, 2 * r:2 * r + 2],
                        mybir.ActivationFunctionType.Sign, bias=bias_t)
                    i_mask += 1
                    if i_mask % MASK_BATCH == 0 or i_mask == n_mask:
                        # The mask drain (~9.3MB) exceeds one DGE ring'"'"'s
                        # sustained rate; alternate rings.  gpsimd'"'"'s ring is
                        # FIFO behind the vt input stream, so route there
                        # only after that stream has drained.
                        idx = (i_mask - 1) // MASK_BATCH
                        eng = nc.gpsimd if (idx >= 3 and idx % 2) else nc.sync
                        eng.dma_start(out=mask_d[idx], in_=stage)
                        stage = None
                else:
                    if mxt is None:
                        mxt = mx_pool.tile(
                            [128, MX_BATCH, 2, nseg_h], mybir.dt.float32)
                    b = i_max % MX_BATCH
                    nc.vector.tensor_reduce(
                        mxt[:, b], ps[:, 2 * r:2 * r + 2],
                        axis=mybir.AxisListType.X, op=mybir.AluOpType.max)
                    i_max += 1
                    if i_max % MX_BATCH == 0 or i_max == n_max:
                        nc.sync.dma_start(
                            out=mx_d[(i_max - 1) // MX_BATCH], in_=mxt)
                        mxt = None
    nc.finalize()
    return nc


def _device_candidate_edges(V32: np.ndarray, thr: float):
    """Run the SPMD kernel on 8 cores; return candidate pairs (ci, cj) with
    sims_fp8 >= thr - EPS, restricted to the computed upper-triangle blocks
    (a superset of every pair the reference's column mask admits).  Vector
    (reduce-max) units contribute whole 128-col segments per flagged row."""
    global LAST_EXEC_NS
    n = V32.shape[0]
    thr_dev = float(thr) - EPS

    key = (n, round(thr_dev, 9))
    if key not in _BUILD_CACHE:
        _BUILD_CACHE[key] = _build_program(n, thr_dev)
    nc = _BUILD_CACHE[key]

    vt8 = np.ascontiguousarray(
        V32.T.reshape(2, 128, n).astype(ml_dtypes.float8_e4m3))
    in_maps = []
    for c in range(N_CORES):
        cols = np.concatenate([
            np.arange(I_TILE * _itile_for_slot(c, s),
                      I_TILE * (_itile_for_slot(c, s) + 1))
            for s in range(SLOTS)])
        vq8 = np.ascontiguousarray(vt8[:, :, cols])
        in_maps.append({"vt": vt8, "vq": vq8})

    if TRACE:
        _ensure_ntff_hook()
    res = run_bass_kernel_spmd(
        nc, in_maps, core_ids=list(range(N_CORES)), trace=TRACE)
    if TRACE:
        LAST_EXEC_NS = res.exec_time_ns

    units = _unit_layout(n // UNIT)
    kinds = _assign_engines(units)
    n_max = sum(1 for k in kinds if k == "max")
    ci_all, cj_all = [], []
    for c in range(N_CORES):
        o_mask = res.results[c]["mask"]  # [nb_mask, 128, MASK_BATCH, 1024]
        o_mx = res.results[c]["mx"]      # [nb_max, 128, MX_BATCH, 8]
        t_for_s = np.array([_itile_for_slot(c, s) for s in range(SLOTS)],
                           dtype=np.int64)
        i_mask = i_max = 0
        for (s, jb), kind in zip(units, kinds):
            if kind == "mask":
                o = o_mask[i_mask // MASK_BATCH][:, i_mask % MASK_BATCH]
                o = o.reshape(I_TILE, UNIT)
                i_mask += 1
                bp, bq = np.nonzero(o == 1)
                if bp.size:
                    ci_all.append(I_TILE * t_for_s[s] + bp)
                    cj_all.append(UNIT * jb + bq)
            else:
                m = o_mx[i_max // MX_BATCH][:, i_max % MX_BATCH]
                m = m.reshape(I_TILE, UNIT // SEG)
                i_max += 1
                bp, bs = np.nonzero(m >= thr_dev)
                if bp.size:
                    # expand each flagged segment to its SEG columns
                    ci_all.append(np.repeat(I_TILE * t_for_s[s] + bp, SEG))
                    cj_all.append(
                        (UNIT * jb + bs[:, None] * SEG
                         + np.arange(SEG)[None, :]).reshape(-1))
    if not ci_all:
        return (np.zeros(0, np.int64), np.zeros(0, np.int64))
    return np.concatenate(ci_all), np.concatenate(cj_all)


def _exact_edges(V32, ci, cj, thr, B):
    """From candidate pairs, produce exact reference edges:
    fp32 sims >= thr and j >= (i//B)*B + 1.  Returns (ci, cj)."""
    keep = cj >= (ci // B) * B + 1
    ci, cj = ci[keep], cj[keep]
    if ci.size:
        sims = np.empty(ci.size, np.float32)
        CH = 1 << 19
        for lo in range(0, ci.size, CH):
            hi = min(lo + CH, ci.size)
            sims[lo:hi] = np.einsum(
                "ij,ij->i", V32[ci[lo:hi]], V32[cj[lo:hi]])
        keep = sims >= np.float32(thr)
        ci, cj = ci[keep], cj[keep]
    return ci, cj


def _merge_replay(g, ci, cj, B):
    """Faithful replay of the reference's sequential merge.

    Per batch: the matched sets are frozen at batch start (with the
    g_i0 != g_j filter evaluated on batch-start group ids), then rows are
    processed sequentially; each row i merges every row whose CURRENT group
    id appears among the CURRENT group ids of its matched j's into i's
    CURRENT group."""
    n = g.shape[0]
    if ci.size == 0:
        return g
    order = np.argsort(ci, kind="stable")
    ci, cj = ci[order], cj[order]
    row_ids, row_starts = np.unique(ci, return_index=True)
    row_ends = np.append(row_starts[1:], ci.size)
    row_j = {int(i): cj[s:e] for i, s, e in zip(row_ids, row_starts, row_ends)}

    flag = np.zeros(max(n, int(g.max()) + 1), dtype=bool)
    for b in np.unique(row_ids // B):
        bs = int(b) * B
        g0 = g.copy()
        frozen = []
        for i in range(bs, bs + B):
            J = row_j.get(i)
            if J is None:
                continue
            J = J[g0[J] != g0[i]]
            if J.size:
                frozen.append((i, J))
        for i, J in frozen:
            mg = np.unique(g[J])
            flag[mg] = True
            sel = flag[g]
            g[sel] = g[i]
            flag[mg] = False
    return g


def kernel(V, group_ids, cos_threshold, batch_size):
    V32 = np.ascontiguousarray(np.asarray(V, dtype=np.float32))
    g = np.asarray(group_ids, dtype=np.int32).copy()
    thr = float(np.asarray(cos_threshold).reshape(-1)[0])
    B = int(np.asarray(batch_size))

    ci, cj = _device_candidate_edges(V32, thr)
    ci, cj = _exact_edges(V32, ci, cj, thr, B)
    g = _merge_replay(g, ci, cj, B)
    return g.astype(np.int32)
